# revision 5
# baseline (speedup 1.0000x reference)
"""Trainium2 Bass kernel for the attention problem.

B=2, S=2048, D=1024, NH=16, HD=64. Returns (out[B,S,D], scores[B,NH,S,S])
where scores are PRE-softmax: q@k^T*scale + at + mask.

Sharding: 8 cores, core i handles batch b=i//4 and the 4 heads
[ (i%4)*4, (i%4)*4+4 ).  No collectives: each core emits its 4 heads'
scores plus a partial output projection (Wo input-dim shard); the host
sums the 4 partials per batch and adds bo.

Device pipeline per core (all statically unrolled, Tile-scheduled):
  A) hs [S,D] -> hsT chunks [128din, S] via PE transpose (f32) + copies.
  B) QT/KT = W @ hsT (float32r matmuls, scale 1/8 pre-folded into Wq),
     VT likewise; VT -> V [S, 256] bf16 via PE transpose.
  C) per (q-group of 512, head): scores psum = QT^T-slice x KT (f32r);
     DVE adds `at` tile from HBM -> scores sbuf -> DMA out;
     ACT exp -> probs bf16 + accum_out rowsum; recip -> diag(recip) bf16;
     PE "normalizing transpose": P^T*diag via regular bf16 matmuls;
     PV: ctxT[dh,q] += V-block^T x PT (bf16).
  D) out partial = ctxT^T x WoT (float32r) -> DMA out.
"""

import contextlib
import sys

if "/opt/trn_rl_repo" not in sys.path:
    sys.path.insert(0, "/opt/trn_rl_repo")

import ml_dtypes
import numpy as np

from concourse import bacc, mybir, tile
from concourse.bass_utils import run_bass_kernel_spmd

B, S, D, NH = 2, 2048, 1024, 16
HD = D // NH          # 64
NCORES = 8
LH = 4                # local heads per core
LD = LH * HD          # 256 local head dims
SCALE = 1.0 / 8.0     # 1/sqrt(HD), exact power of two

F32 = mybir.dt.float32
F32R = mybir.dt.float32r
BF16 = mybir.dt.bfloat16
IDENT = mybir.ActivationFunctionType.Identity

LAST_RUN = None
_CACHED_NC = None

NQC = S // 128            # 16 q-chunks of 128 rows
NKC = S // 128            # 16 k-chunks
NDC = D // 128            # 8 din chunks


def _build():
    nc = bacc.Bacc("TRN2", target_bir_lowering=False, debug=False,
                   num_devices=NCORES)

    hs_d = nc.dram_tensor("hs", [S, D], F32, kind="ExternalInput")
    at_d = nc.dram_tensor("at4", [LH, S, S], F32, kind="ExternalInput")
    wqt_d = nc.dram_tensor("wqt", [D, LD], F32, kind="ExternalInput")
    wkt_d = nc.dram_tensor("wkt", [D, LD], F32, kind="ExternalInput")
    wvt_d = nc.dram_tensor("wvt", [D, LD], F32, kind="ExternalInput")
    wot_d = nc.dram_tensor("wot", [LD, D], F32, kind="ExternalInput")
    bq_d = nc.dram_tensor("bq", [LD, 1], F32, kind="ExternalInput")
    bk_d = nc.dram_tensor("bk", [LD, 1], F32, kind="ExternalInput")
    bv_d = nc.dram_tensor("bv", [LD, 1], F32, kind="ExternalInput")
    idf_d = nc.dram_tensor("identf", [128, 128], F32, kind="ExternalInput")
    idb_d = nc.dram_tensor("identb", [128, 128], BF16, kind="ExternalInput")

    outp_d = nc.dram_tensor("out_p", [S, D], F32, kind="ExternalOutput")
    sc_d = nc.dram_tensor("scores4", [LH, S, S], F32, kind="ExternalOutput")

    with tile.TileContext(nc) as tc, contextlib.ExitStack() as ctx:
        const_p = ctx.enter_context(tc.tile_pool(name="const", bufs=1))
        identf = const_p.tile([128, 128], F32)
        identb = const_p.tile([128, 128], BF16)
        nc.sync.dma_start(identf[:], idf_d[:])
        nc.sync.dma_start(identb[:], idb_d[:])

        # ---- long-lived pools (survive into stage C/D) -----------------
        wo_p = ctx.enter_context(tc.tile_pool(name="wop", bufs=1))
        wo = [wo_p.tile([128, D], F32R, tag=f"wor{dc}", name=f"wor{dc}")
              for dc in range(2)]
        proj_p = ctx.enter_context(tc.tile_pool(name="proj", bufs=1))
        QT = [proj_p.tile([128, S], F32R, tag=f"QT{dc}", name=f"QT{dc}")
              for dc in range(2)]
        KT = [proj_p.tile([128, S], F32R, tag=f"KT{dc}", name=f"KT{dc}")
              for dc in range(2)]
        v_p = ctx.enter_context(tc.tile_pool(name="vnat", bufs=1))
        V = [v_p.tile([128, LD], BF16, tag=f"V{kc}", name=f"V{kc}")
             for kc in range(NKC)]

        # ---- stages A+B in a scope whose SBUF frees afterwards ---------
        with tc.tile_pool(name="wts", bufs=1) as w_p, \
             tc.tile_pool(name="hsTp", bufs=1) as hsT_p, \
             tc.tile_pool(name="vtp", bufs=1) as vt_p, \
             tc.tile_pool(name="hsA", bufs=6) as hs_p, \
             tc.tile_pool(name="psAB", bufs=4, space="PSUM") as psB:
            # weights: load f32, convert to f32r
            wq, wk, wv = [], [], []
            for wn, dram, lst in (("q", wqt_d, wq), ("k", wkt_d, wk),
                                  ("v", wvt_d, wv)):
                for jc in range(NDC):
                    t32 = w_p.tile([128, LD], F32, tag="w32", bufs=3,
                                   name=f"w{wn}32_{jc}")
                    nc.sync.dma_start(t32[:], dram[jc * 128:(jc + 1) * 128, :])
                    tr = w_p.tile([128, LD], F32R, tag=f"w{wn}r{jc}",
                                  name=f"w{wn}r{jc}")
                    nc.vector.tensor_copy(tr[:], t32[:])
                    lst.append(tr)
            for dc in range(2):
                t32 = w_p.tile([128, D], F32, tag="wo32", bufs=2,
                               name=f"wo32_{dc}")
                nc.sync.dma_start(t32[:], wot_d[dc * 128:(dc + 1) * 128, :])
                nc.vector.tensor_copy(wo[dc][:], t32[:])
            biases = {}
            for wn, dram in (("q", bq_d), ("k", bk_d), ("v", bv_d)):
                for dc in range(2):
                    bt = w_p.tile([128, 1], F32, tag=f"b{wn}{dc}",
                                  name=f"b{wn}{dc}")
                    nc.sync.dma_start(bt[:], dram[dc * 128:(dc + 1) * 128, :])
                    biases[(wn, dc)] = bt

            # stage A: hsT via PE transpose
            hsT = []
            for jc in range(NDC):
                hsT.append(hsT_p.tile([128, S], F32R, tag=f"hsT{jc}",
                                      name=f"hsT{jc}"))
            for ibg in range(4):          # groups of 4 s-blocks
                hts = []
                for ii in range(4):
                    ib = ibg * 4 + ii
                    ht = hs_p.tile([128, D], F32, tag="ht",
                                   name=f"ht{ibg}_{ii}")
                    nc.sync.dma_start(ht[:], hs_d[ib * 128:(ib + 1) * 128, :])
                    hts.append(ht)
                for jc in range(NDC):
                    pst = psB.tile([128, 512], F32, tag="pst",
                                   name=f"psA{ibg}_{jc}")
                    for ii in range(4):
                        nc.tensor.transpose(
                            pst[:, ii * 128:(ii + 1) * 128],
                            hts[ii][:, jc * 128:(jc + 1) * 128],
                            identf[:])
                    if (ibg * NDC + jc) % 2 == 0:
                        nc.vector.tensor_copy(
                            hsT[jc][:, ibg * 512:(ibg + 1) * 512], pst[:])
                    else:
                        nc.scalar.activation(
                            hsT[jc][:, ibg * 512:(ibg + 1) * 512], pst[:], IDENT)

            # stage B: projections (f32r matmuls)
            VT = [vt_p.tile([128, S], F32, tag=f"VT{dc}", name=f"VT{dc}")
                  for dc in range(2)]
            for wlist, dest, bname in ((wq, QT, "q"), (wk, KT, "k"),
                                       (wv, VT, "v")):
                for dc in range(2):
                    for ss in range(4):
                        pst = psB.tile([128, 512], F32, tag="pst",
                                       name=f"psB{bname}{dc}_{ss}")
                        for jc in range(NDC):
                            nc.tensor.matmul(
                                pst[:],
                                wlist[jc][:, dc * 128:(dc + 1) * 128],
                                hsT[jc][:, ss * 512:(ss + 1) * 512],
                                start=(jc == 0), stop=(jc == NDC - 1))
                        nc.scalar.activation(
                            dest[dc][:, ss * 512:(ss + 1) * 512], pst[:],
                            IDENT, bias=biases[(bname, dc)][:])

            # VT -> V (natural layout, bf16) via PE transpose
            for kc in range(NKC):
                psv = psB.tile([128, 256], F32, tag="psv", name=f"psv{kc}")
                for dc in range(2):
                    nc.tensor.transpose(
                        psv[:, dc * 128:(dc + 1) * 128],
                        VT[dc][:, kc * 128:(kc + 1) * 128],
                        identf[:])
                if kc % 2 == 0:
                    nc.vector.tensor_copy(V[kc][:], psv[:])
                else:
                    nc.scalar.activation(V[kc][:], psv[:], IDENT)

        # ---- stage C + D: attention ------------------------------------
        with tc.tile_pool(name="at", bufs=4) as at_p, \
             tc.tile_pool(name="sc", bufs=3) as sc_p, \
             tc.tile_pool(name="pr", bufs=2) as pr_p, \
             tc.tile_pool(name="pt", bufs=2) as pt_p, \
             tc.tile_pool(name="ctx", bufs=2) as ctx_p, \
             tc.tile_pool(name="osb", bufs=2) as o_p, \
             tc.tile_pool(name="small", bufs=8) as sm_p, \
             tc.tile_pool(name="psS", bufs=2, space="PSUM") as psS, \
             tc.tile_pool(name="psP", bufs=2, space="PSUM") as psP, \
             tc.tile_pool(name="psC", bufs=1, space="PSUM") as psC, \
             tc.tile_pool(name="psO", bufs=1, space="PSUM") as psO:

            copy_flip = 0
            for qg in range(4):
                ctx_sb = [ctx_p.tile([128, 512], F32R, tag=f"ctx{dc}",
                                     name=f"ctx{qg}_{dc}")
                          for dc in range(2)]
                for h in range(LH):
                    dc_h, po_h = h // 2, (h % 2) * 64
                    PT = pt_p.tile([128, NKC, 512], BF16, tag="PT",
                                   name=f"PT{qg}_{h}")
                    for qc in range(4):
                        q0 = qg * 512 + qc * 128
                        at_t = at_p.tile([128, S], F32, tag="at",
                                         name=f"at{qg}_{h}_{qc}")
                        nc.sync.dma_start(at_t[:], at_d[h, q0:q0 + 128, :])
                        sc_t = sc_p.tile([128, S], F32, tag="sc",
                                         name=f"sc{qg}_{h}_{qc}")
                        for hh in range(2):
                            pss = psS.tile([128, 1024], F32, tag="psS",
                                           name=f"psS{qg}_{h}_{qc}_{hh}")
                            for ks in range(2):
                                k0 = hh * 1024 + ks * 512
                                nc.tensor.matmul(
                                    pss[:, ks * 512:(ks + 1) * 512],
                                    QT[dc_h][po_h:po_h + 64, q0:q0 + 128],
                                    KT[dc_h][po_h:po_h + 64, k0:k0 + 512],
                                    start=True, stop=True)
                            nc.vector.tensor_tensor(
                                sc_t[:, hh * 1024:(hh + 1) * 1024],
                                pss[:],
                                at_t[:, hh * 1024:(hh + 1) * 1024],
                                mybir.AluOpType.add)
                        nc.scalar.dma_start(sc_d[h, q0:q0 + 128, :], sc_t[:])
                        pr_t = pr_p.tile([128, S], BF16, tag="pr",
                                         name=f"pr{qg}_{h}_{qc}")
                        rs = sm_p.tile([128, 1], F32, tag="rs",
                                       name=f"rs{qg}_{h}_{qc}")
                        nc.scalar.activation(
                            pr_t[:], sc_t[:],
                            mybir.ActivationFunctionType.Exp,
                            accum_out=rs[:])
                        rc = sm_p.tile([128, 1], F32, tag="rc",
                                       name=f"rc{qg}_{h}_{qc}")
                        nc.vector.reciprocal(rc[:], rs[:])
                        diag = sm_p.tile([128, 128], BF16, tag="diag",
                                         name=f"diag{qg}_{h}_{qc}")
                        nc.vector.tensor_scalar_mul(diag[:], identb[:], rc[:])
                        for kbg in range(4):
                            psp = psP.tile([128, 4, 128], F32, tag="psP",
                                           name=f"psP{qg}_{h}_{qc}_{kbg}")
                            for kk in range(4):
                                kb = kbg * 4 + kk
                                nc.tensor.matmul(
                                    psp[:, kk, :],
                                    pr_t[:, kb * 128:(kb + 1) * 128],
                                    diag[:],
                                    start=True, stop=True)
                            dst = PT[:, kbg * 4:(kbg + 1) * 4,
                                     qc * 128:(qc + 1) * 128]
                            if copy_flip % 2 == 0:
                                nc.vector.tensor_copy(dst, psp[:])
                            else:
                                nc.scalar.activation(dst, psp[:], IDENT)
                            copy_flip += 1
                    # PV for this (qg, h)
                    psc = psC.tile([64, 512], F32, tag="psC",
                                   name=f"psC{qg}_{h}")
                    for kc in range(NKC):
                        nc.tensor.matmul(
                            psc[:],
                            V[kc][:, h * HD:(h + 1) * HD],
                            PT[:, kc, :],
                            start=(kc == 0), stop=(kc == NKC - 1))
                    nc.scalar.activation(
                        ctx_sb[dc_h][po_h:po_h + 64, :], psc[:], IDENT)

                # ---- stage D: output projection for this q-group --------
                for sb in range(4):
                    s0 = qg * 512 + sb * 128
                    osb = o_p.tile([128, D], F32, tag="osb",
                                   name=f"osb{qg}_{sb}")
                    for n2 in range(2):
                        pso = psO.tile([128, 512], F32, tag="psO",
                                       name=f"psO{qg}_{sb}_{n2}")
                        for dc in range(2):
                            nc.tensor.matmul(
                                pso[:],
                                ctx_sb[dc][:, sb * 128:(sb + 1) * 128],
                                wo[dc][:, n2 * 512:(n2 + 1) * 512],
                                start=(dc == 0), stop=(dc == 1))
                        if sb % 2 == 0:
                            nc.vector.tensor_copy(
                                osb[:, n2 * 512:(n2 + 1) * 512], pso[:])
                        else:
                            nc.scalar.activation(
                                osb[:, n2 * 512:(n2 + 1) * 512], pso[:], IDENT)
                    nc.gpsimd.dma_start(outp_d[s0:s0 + 128, :], osb[:])

    nc.compile()
    return nc


def _shard_inputs(inputs):
    hs = np.asarray(inputs["hidden_states"], dtype=np.float32)
    mask = np.asarray(inputs["attention_mask"], dtype=np.float32)
    at = np.asarray(inputs["at"], dtype=np.float32)
    Wq = np.asarray(inputs["Wq"], dtype=np.float32)
    bq = np.asarray(inputs["bq"], dtype=np.float32)
    Wk = np.asarray(inputs["Wk"], dtype=np.float32)
    bk = np.asarray(inputs["bk"], dtype=np.float32)
    Wv = np.asarray(inputs["Wv"], dtype=np.float32)
    bv = np.asarray(inputs["bv"], dtype=np.float32)
    Wo = np.asarray(inputs["Wo"], dtype=np.float32)

    identf = np.eye(128, dtype=np.float32)
    identb = np.eye(128, dtype=np.float32).astype(ml_dtypes.bfloat16)

    in_maps = []
    for core in range(NCORES):
        b, hg = core // 4, core % 4
        r0 = hg * LD
        rows = slice(r0, r0 + LD)
        in_maps.append({
            "hs": np.ascontiguousarray(hs[b]),
            "at4": np.ascontiguousarray(
                at[b, hg * LH:(hg + 1) * LH]
                + mask[b, 0, 0, :][None, None, :]),
            "wqt": np.ascontiguousarray((Wq[rows, :] * SCALE).T),
            "wkt": np.ascontiguousarray(Wk[rows, :].T),
            "wvt": np.ascontiguousarray(Wv[rows, :].T),
            "wot": np.ascontiguousarray(Wo[:, rows].T),
            "bq": np.ascontiguousarray((bq[rows] * SCALE).reshape(LD, 1)),
            "bk": np.ascontiguousarray(bk[rows].reshape(LD, 1)),
            "bv": np.ascontiguousarray(bv[rows].reshape(LD, 1)),
            "identf": identf,
            "identb": identb,
        })
    return in_maps


def kernel(**inputs):
    global LAST_RUN, _CACHED_NC
    if _CACHED_NC is None:
        _CACHED_NC = _build()
    nc = _CACHED_NC
    in_maps = _shard_inputs(inputs)
    res = run_bass_kernel_spmd(nc, in_maps, core_ids=list(range(NCORES)))
    LAST_RUN = res

    bo = np.asarray(inputs["bo"], dtype=np.float32)
    out = np.zeros((B, S, D), dtype=np.float32)
    scores = np.empty((B, NH, S, S), dtype=np.float32)
    for core in range(NCORES):
        b, hg = core // 4, core % 4
        scores[b, hg * LH:(hg + 1) * LH] = res.results[core]["scores4"]
        out[b] += res.results[core]["out_p"]
    out += bo[None, None, :]
    return out, scores


# revision 10
# speedup vs baseline: 1.2129x; 1.2129x over previous
"""Trainium2 Bass kernel for the attention problem.

B=2, S=2048, D=1024, NH=16, HD=64. Returns (out[B,S,D], scores[B,NH,S,S])
where scores are PRE-softmax: q@k^T*scale + at + mask.

Sharding: 8 cores, core i handles batch b=i//4 and the 4 heads
[ (i%4)*4, (i%4)*4+4 ).  No collectives: each core emits its 4 heads'
scores plus a partial output projection (Wo input-dim shard); the host
sums the 4 partials per batch and adds bo.

v2 design (all matmuls bf16, transpose-free attention):
  A) hs -> bf16 -> hsT chunks [128din, S] via PE transpose.
  B) projections (K=128 bf16 matmuls); Q/K written both packed
     (QTb/KTb, 2 heads per 128 rows) and per-head zero-padded
     (QTzb/KTzb, other head's 64 rows zeroed) so every attention matmul
     contracts over K=128; V -> natural layout with an appended ones
     column per head (V_aug [128, 4, 65]).
  C) per (q-group 512, head):
     natural scores psum = QTzb_h-slice x KTb  -> DVE +at -> DMA out;
     ST psum = KTzb_h-block x QTb (scores transposed, recomputed);
     PT = exp(ST psum) via ACT straight into SBUF bf16;
     PV: psc[65,512] += V_aug_h^T x PT  (row 64 = exp row-sums);
     recip = 1/psc[64]; gpsimd broadcasts it; DVE mult -> ctx_sb bf16.
  D) out partial = ctx_sb x WoT -> DMA out.
"""

import contextlib
import sys

if "/opt/trn_rl_repo" not in sys.path:
    sys.path.insert(0, "/opt/trn_rl_repo")

import ml_dtypes
import numpy as np

from concourse import bacc, mybir, tile
from concourse.bass_utils import run_bass_kernel_spmd

B, S, D, NH = 2, 2048, 1024, 16
HD = D // NH          # 64
NCORES = 8
LH = 4                # local heads per core
LD = LH * HD          # 256 local head dims
SCALE = 1.0 / 8.0     # 1/sqrt(HD), exact power of two

F32 = mybir.dt.float32
BF16 = mybir.dt.bfloat16
IDENT = mybir.ActivationFunctionType.Identity
EXP = mybir.ActivationFunctionType.Exp

LAST_RUN = None
_CACHED_NC = None

NKC = S // 128            # 16 k-chunks
NDC = D // 128            # 8 din chunks


def _build():
    nc = bacc.Bacc("TRN2", target_bir_lowering=False, debug=False,
                   num_devices=NCORES)

    hs_d = nc.dram_tensor("hs", [S, D], F32, kind="ExternalInput")
    at_d = nc.dram_tensor("at4T", [LH, S, S], F32, kind="ExternalInput")
    wqt_d = nc.dram_tensor("wqt", [D, LD], F32, kind="ExternalInput")
    wkt_d = nc.dram_tensor("wkt", [D, LD], F32, kind="ExternalInput")
    wvt_d = nc.dram_tensor("wvt", [D, LD], F32, kind="ExternalInput")
    wot_d = nc.dram_tensor("wot", [LD, D], F32, kind="ExternalInput")
    bq_d = nc.dram_tensor("bq", [LD, 1], F32, kind="ExternalInput")
    bk_d = nc.dram_tensor("bk", [LD, 1], F32, kind="ExternalInput")
    bv_d = nc.dram_tensor("bv", [LD, 1], F32, kind="ExternalInput")
    idb_d = nc.dram_tensor("identb", [128, 128], BF16, kind="ExternalInput")

    outp_d = nc.dram_tensor("out_p", [S, D], F32, kind="ExternalOutput")
    sc_d = nc.dram_tensor("scoresT4", [LH, S, S], F32, kind="ExternalOutput")

    with tile.TileContext(nc) as tc, contextlib.ExitStack() as ctx:
        const_p = ctx.enter_context(tc.tile_pool(name="const", bufs=1))
        identb = const_p.tile([128, 128], BF16)
        nc.sync.dma_start(identb[:], idb_d[:])

        # ---- long-lived pools (survive into stage C/D) -----------------
        wo_p = ctx.enter_context(tc.tile_pool(name="wop", bufs=1))
        wo = [wo_p.tile([128, D], BF16, tag=f"wob{dc}", name=f"wob{dc}")
              for dc in range(2)]
        proj_p = ctx.enter_context(tc.tile_pool(name="proj", bufs=1))
        QTb = [proj_p.tile([128, S], BF16, tag=f"QTb{dc}", name=f"QTb{dc}")
               for dc in range(2)]
        KTz = [proj_p.tile([128, S], BF16, tag=f"KTz{h}", name=f"KTz{h}")
               for h in range(LH)]
        v_p = ctx.enter_context(tc.tile_pool(name="vnat", bufs=1))
        V = [v_p.tile([128, LH, HD + 1], BF16, tag=f"V{kc}", name=f"V{kc}")
             for kc in range(NKC)]

        # zero the padded K tiles and set the V ones-columns (gpsimd)
        for h in range(LH):
            nc.gpsimd.memset(KTz[h][:], 0.0)
        for kc in range(NKC):
            nc.gpsimd.memset(V[kc][:, :, HD:HD + 1], 1.0)

        # ---- stages A+B in a scope whose SBUF frees afterwards ---------
        with tc.tile_pool(name="wts", bufs=1) as w_p, \
             tc.tile_pool(name="hsTp", bufs=1) as hsT_p, \
             tc.tile_pool(name="vtp", bufs=1) as vt_p, \
             tc.tile_pool(name="hsA", bufs=6) as hs_p, \
             tc.tile_pool(name="psAB", bufs=4, space="PSUM") as psB:
            # weights: load f32, convert to bf16
            wq, wk, wv = [], [], []
            for wn, dram, lst in (("q", wqt_d, wq), ("k", wkt_d, wk),
                                  ("v", wvt_d, wv)):
                for jc in range(NDC):
                    t32 = w_p.tile([128, LD], F32, tag="w32", bufs=3,
                                   name=f"w{wn}32_{jc}")
                    nc.sync.dma_start(t32[:], dram[jc * 128:(jc + 1) * 128, :])
                    tr = w_p.tile([128, LD], BF16, tag=f"w{wn}b{jc}",
                                  name=f"w{wn}b{jc}")
                    nc.vector.tensor_copy(tr[:], t32[:])
                    lst.append(tr)
            for dc in range(2):
                t32 = w_p.tile([128, D], F32, tag="wo32", bufs=2,
                               name=f"wo32_{dc}")
                nc.sync.dma_start(t32[:], wot_d[dc * 128:(dc + 1) * 128, :])
                nc.vector.tensor_copy(wo[dc][:], t32[:])
            biases = {}
            for wn, dram in (("q", bq_d), ("k", bk_d), ("v", bv_d)):
                for dc in range(2):
                    bt = w_p.tile([128, 1], F32, tag=f"b{wn}{dc}",
                                  name=f"b{wn}{dc}")
                    nc.sync.dma_start(bt[:], dram[dc * 128:(dc + 1) * 128, :])
                    biases[(wn, dc)] = bt

            # stage A: hs -> bf16 -> hsT via PE transpose
            hsT = []
            for jc in range(NDC):
                hsT.append(hsT_p.tile([128, S], BF16, tag=f"hsT{jc}",
                                      name=f"hsT{jc}"))
            for ibg in range(4):          # groups of 4 s-blocks
                hts = []
                for ii in range(4):
                    ib = ibg * 4 + ii
                    ht = hs_p.tile([128, D], F32, tag="ht",
                                   name=f"ht{ibg}_{ii}")
                    nc.sync.dma_start(ht[:], hs_d[ib * 128:(ib + 1) * 128, :])
                    htb = hs_p.tile([128, D], BF16, tag="htb",
                                    name=f"htb{ibg}_{ii}")
                    nc.vector.tensor_copy(htb[:], ht[:])
                    hts.append(htb)
                for jc in range(NDC):
                    pst = psB.tile([128, 512], BF16, tag="pstA", bufs=2,
                                   name=f"psA{ibg}_{jc}")
                    for ii in range(4):
                        nc.tensor.transpose(
                            pst[:, ii * 128:(ii + 1) * 128],
                            hts[ii][:, jc * 128:(jc + 1) * 128],
                            identb[:])
                    if (ibg * NDC + jc) % 2 == 0:
                        nc.vector.tensor_copy(
                            hsT[jc][:, ibg * 512:(ibg + 1) * 512], pst[:])
                    else:
                        nc.scalar.activation(
                            hsT[jc][:, ibg * 512:(ibg + 1) * 512], pst[:], IDENT)

            # stage B: projections (bf16, K=128)
            VT = [vt_p.tile([128, S], BF16, tag=f"VT{dc}", name=f"VT{dc}")
                  for dc in range(2)]
            for wlist, bname in ((wk, "k"), (wq, "q"), (wv, "v")):
                for dc in range(2):
                    for ss in range(4):
                        pst = psB.tile([128, 512], F32, tag="pst",
                                       name=f"psB{bname}{dc}_{ss}")
                        for jc in range(NDC):
                            nc.tensor.matmul(
                                pst[:],
                                wlist[jc][:, dc * 128:(dc + 1) * 128],
                                hsT[jc][:, ss * 512:(ss + 1) * 512],
                                start=(jc == 0), stop=(jc == NDC - 1))
                        sl = slice(ss * 512, (ss + 1) * 512)
                        bias = biases[(bname, dc)]
                        if bname == "v":
                            nc.scalar.activation(VT[dc][:, sl], pst[:],
                                                 IDENT, bias=bias[:])
                        elif bname == "q":
                            nc.scalar.activation(QTb[dc][:, sl], pst[:],
                                                 IDENT, bias=bias[:])
                        else:
                            for hh in range(2):
                                h = dc * 2 + hh
                                po = hh * 64
                                nc.scalar.activation(
                                    KTz[h][po:po + 64, sl],
                                    pst[po:po + 64, :],
                                    IDENT, bias=bias[po:po + 64, :])

            # VT -> V_aug (natural layout + ones column) via PE transpose
            for kc in range(NKC):
                psv = psB.tile([128, 256], BF16, tag="psv", bufs=2, name=f"psv{kc}")
                for dc in range(2):
                    nc.tensor.transpose(
                        psv[:, dc * 128:(dc + 1) * 128],
                        VT[dc][:, kc * 128:(kc + 1) * 128],
                        identb[:])
                dst = V[kc][:, :, 0:HD]
                if kc % 2 == 0:
                    nc.vector.tensor_copy(dst, psv[:])
                else:
                    nc.scalar.activation(dst, psv[:], IDENT)

        # ---- stage C + D: attention ------------------------------------
        with tc.tile_pool(name="at", bufs=12) as at_p, \
             tc.tile_pool(name="st", bufs=6) as st_p, \
             tc.tile_pool(name="pt", bufs=12) as pt_p, \
             tc.tile_pool(name="ctx", bufs=2) as ctx_p, \
             tc.tile_pool(name="osb", bufs=2) as o_p, \
             tc.tile_pool(name="small", bufs=6) as sm_p, \
             tc.tile_pool(name="psT", bufs=3, space="PSUM") as psT, \
             tc.tile_pool(name="psC", bufs=1, space="PSUM") as psC, \
             tc.tile_pool(name="psO", bufs=1, space="PSUM") as psO:

            for qg in range(4):
                qsl = slice(qg * 512, (qg + 1) * 512)
                ctx_sb = [ctx_p.tile([128, 512], BF16, tag=f"ctx{dc}",
                                     name=f"ctx{qg}_{dc}")
                          for dc in range(2)]
                for h in range(LH):
                    dc_h, po_h = h // 2, (h % 2) * 64
                    # transposed scores (+atT) -> exp -> PT tiles (bf16)
                    PT = []
                    for kg in range(8):       # pairs of k-blocks
                        at_t = at_p.tile([128, 2, 512], F32, tag="at",
                                         name=f"at{qg}_{h}_{kg}")
                        nc.sync.dma_start(
                            at_t[:],
                            at_d[h, kg * 256:(kg + 1) * 256, qsl]
                            .rearrange("(kk p) q -> p kk q", p=128))
                        pst2 = psT.tile([128, 1024], F32, tag="psT",
                                        name=f"psT{qg}_{h}_{kg}")
                        for kk in range(2):
                            kb = kg * 2 + kk
                            nc.tensor.matmul(
                                pst2[:, kk * 512:(kk + 1) * 512],
                                KTz[h][:, kb * 128:(kb + 1) * 128],
                                QTb[dc_h][:, qsl],
                                start=True, stop=True)
                        st_t = st_p.tile([128, 2, 512], F32, tag="st",
                                         name=f"st{qg}_{h}_{kg}")
                        nc.vector.tensor_tensor(
                            st_t[:], pst2[:], at_t[:],
                            mybir.AluOpType.add)
                        nc.scalar.dma_start(
                            sc_d[h, kg * 256:(kg + 1) * 256, qsl]
                            .rearrange("(kk p) q -> p kk q", p=128),
                            st_t[:])
                        ptt = pt_p.tile([128, 1024], BF16, tag="PT",
                                        name=f"PT{qg}_{h}_{kg}")
                        nc.scalar.activation(ptt[:], st_t[:], EXP)
                        PT.append(ptt)

                    # PV with ones row: psc[0:64]=ctx^T, psc[64]=rowsum
                    psc = psC.tile([65, 512], F32, tag="psC",
                                   name=f"psC{qg}_{h}")
                    for kg in range(8):
                        for kk in range(2):
                            kc = kg * 2 + kk
                            nc.tensor.matmul(
                                psc[:],
                                V[kc][:, h, :],
                                PT[kg][:, kk * 512:(kk + 1) * 512],
                                start=(kc == 0), stop=(kc == NKC - 1))
                    rs_sb = sm_p.tile([1, 512], F32, tag="rs",
                                      name=f"rs{qg}_{h}")
                    nc.scalar.activation(rs_sb[:], psc[64:65, :], IDENT)
                    rc_sb = sm_p.tile([1, 512], F32, tag="rc",
                                      name=f"rc{qg}_{h}")
                    nc.vector.reciprocal(rc_sb[:], rs_sb[:])
                    rcb = sm_p.tile([64, 512], F32, tag="rcb",
                                    name=f"rcb{qg}_{h}")
                    nc.gpsimd.partition_broadcast(rcb[:], rc_sb[:])
                    nc.vector.tensor_tensor(
                        ctx_sb[dc_h][po_h:po_h + 64, :],
                        psc[0:64, :], rcb[:],
                        mybir.AluOpType.mult)

                # ---- stage D: output projection for this q-group --------
                for sb in range(4):
                    s0 = qg * 512 + sb * 128
                    osb = o_p.tile([128, D], F32, tag="osb",
                                   name=f"osb{qg}_{sb}")
                    for n2 in range(2):
                        pso = psO.tile([128, 512], F32, tag="psO",
                                       name=f"psO{qg}_{sb}_{n2}")
                        for dc in range(2):
                            nc.tensor.matmul(
                                pso[:],
                                ctx_sb[dc][:, sb * 128:(sb + 1) * 128],
                                wo[dc][:, n2 * 512:(n2 + 1) * 512],
                                start=(dc == 0), stop=(dc == 1))
                        if sb % 2 == 0:
                            nc.vector.tensor_copy(
                                osb[:, n2 * 512:(n2 + 1) * 512], pso[:])
                        else:
                            nc.scalar.activation(
                                osb[:, n2 * 512:(n2 + 1) * 512], pso[:], IDENT)
                    nc.gpsimd.dma_start(outp_d[s0:s0 + 128, :], osb[:])

    nc.compile()
    return nc


def _shard_inputs(inputs):
    hs = np.asarray(inputs["hidden_states"], dtype=np.float32)
    mask = np.asarray(inputs["attention_mask"], dtype=np.float32)
    at = np.asarray(inputs["at"], dtype=np.float32)
    Wq = np.asarray(inputs["Wq"], dtype=np.float32)
    bq = np.asarray(inputs["bq"], dtype=np.float32)
    Wk = np.asarray(inputs["Wk"], dtype=np.float32)
    bk = np.asarray(inputs["bk"], dtype=np.float32)
    Wv = np.asarray(inputs["Wv"], dtype=np.float32)
    bv = np.asarray(inputs["bv"], dtype=np.float32)
    Wo = np.asarray(inputs["Wo"], dtype=np.float32)

    identb = np.eye(128, dtype=np.float32).astype(ml_dtypes.bfloat16)

    in_maps = []
    for core in range(NCORES):
        b, hg = core // 4, core % 4
        r0 = hg * LD
        rows = slice(r0, r0 + LD)
        in_maps.append({
            "hs": np.ascontiguousarray(hs[b]),
            "at4T": np.ascontiguousarray(
                at[b, hg * LH:(hg + 1) * LH].transpose(0, 2, 1)
                + mask[b, 0, 0, :][None, :, None]),
            "wqt": np.ascontiguousarray((Wq[rows, :] * SCALE).T),
            "wkt": np.ascontiguousarray(Wk[rows, :].T),
            "wvt": np.ascontiguousarray(Wv[rows, :].T),
            "wot": np.ascontiguousarray(Wo[:, rows].T),
            "bq": np.ascontiguousarray((bq[rows] * SCALE).reshape(LD, 1)),
            "bk": np.ascontiguousarray(bk[rows].reshape(LD, 1)),
            "bv": np.ascontiguousarray(bv[rows].reshape(LD, 1)),
            "identb": identb,
        })
    return in_maps


def kernel(**inputs):
    global LAST_RUN, _CACHED_NC
    if _CACHED_NC is None:
        _CACHED_NC = _build()
    nc = _CACHED_NC
    in_maps = _shard_inputs(inputs)
    res = run_bass_kernel_spmd(nc, in_maps, core_ids=list(range(NCORES)))
    LAST_RUN = res

    bo = np.asarray(inputs["bo"], dtype=np.float32)
    out = np.zeros((B, S, D), dtype=np.float32)
    scores = np.empty((B, NH, S, S), dtype=np.float32)
    for core in range(NCORES):
        b, hg = core // 4, core % 4
        scores[b, hg * LH:(hg + 1) * LH] = \
            res.results[core]["scoresT4"].transpose(0, 2, 1)
        out[b] += res.results[core]["out_p"]
    out += bo[None, None, :]
    return out, scores
